# revision 21
# baseline (speedup 1.0000x reference)
"""ABMIL attention-pooling kernel for 8 TRN2 NeuronCores (Bass/Tile).

Reference computation (per bag b of B=4, N=20000 instances, 1024 feats):
    h   = x @ W_pe + b_pe                    [N, 512]
    A_V = tanh(h @ W_V + b_V)                [N, 128]
    A_U = sigmoid(h @ W_U + b_U)             [N, 128]
    a   = (A_V * A_U) @ w_att + b_att        [N, 1]
    A   = softmax(a, axis=0)
    M   = sum(A * h, axis=0)                 [512]
    out = M @ W_cls + b_cls                  [2]

Algebraic rewrites used:
  * classifier-before-pooling: out = (sum_n w_n * (h_n @ W_cls)) / (sum_n w_n)
    + b_cls with w_n = exp(a_n) (softmax shift skipped: logits are O(1)), so
    each core only emits partial (wc[2], s) over its instance shard.
  * sigmoid(y) = (1 + tanh(y/2))/2 so tanh/exp/identity are the only ACT
    functions -> one ACT table set (exp_and_others), no ~2.7us table swaps.
    The 1/2 factors fold into w_att (host) and the U-branch bias.

Sharding: core c -> bag c//2, instance half c%2 (10000 instances each);
the host sums the two partials per bag and applies /s + b_cls.

Device dataflow is fully "transposed" (embedding dim on partitions) so the
x tile loaded as [feat, inst] feeds every matmul with zero transposes:
    hT  [512, n]  = W_pe.T @ xT       (lhsT = W_pe chunks)
    AVT [128, n]  = W_V.T @ hT         etc.
    logit [2, n]  = w_att2.T @ G       (w_att duplicated to 2 rows = free
                                        broadcast of w over 2 partitions)
    cT  [2, n]    = W_cls.T @ hT
    s, wc         = free-dim reductions of exp(logit) and exp(logit)*cT

Compute dtype is float16 (same TensorE rate as bf16, ~5x less quantization
error; all values here are O(1) so fp16 range is safe).  PSUM accumulation
and reductions stay float32.

Engine discipline: TRN2 compute instructions encode exactly ONE semaphore
wait.  The dataflow is arranged so every instruction depends on at most one
other engine: PE reads SBUF tiles produced by DMA or DVE; DVE alone reads
PSUM (copy/cast + bias); ACT alone reads DVE-produced SBUF tiles; DVE
combines ACT outputs.  All weights ship as one packed DMA (one semaphore),
all biases as another, each absorbed by single-purpose preamble ops.
"""

import os
import sys

import numpy as np

# Shapes for this problem (hardcoded per the task contract).
B = 4
N = 20000
IN_DIM = 1024
EMB = 512
ATT = 128
NCLS = 2
N_CORES = 8
N_SHARD = (B * N) // N_CORES  # 10000 instances per core
TILE = 500                    # instances per device tile
N_TILES = N_SHARD // TILE     # 20
KC = IN_DIM // 128            # 8 feature chunks
MC = EMB // 128               # 4 embedding chunks

# packed weight layout (f16, [128, WPACK_COLS]):
#   [0:4096)        W_pe   as [ki, ko*512+e]
#   [4096:4608)     W_V    as [mi, mo*128+a]
#   [4608:5120)     W_U    as [mi, mo*128+a]
#   [5120:5122)     w_att2 (0.5*w_att duplicated to 2 cols)
#   [5122:5130)     W_cls  as [mi, mo*2+o]
W_PE_OFF = 0
W_V_OFF = 4096
W_U_OFF = 4608
W_ATT_OFF = 5120
W_CLS_OFF = 5122
WPACK_COLS = 5130

# packed bias layout (f32, [128, 7]):
#   [0:4) b_pe chunks; [4] b_V; [5] 0.5*b_U; [6] b_att (broadcast all rows)
BPACK_COLS = 7

_cache = {}


def _import_concourse():
    for p in ("/opt/trn_rl_repo", "/root/.axon_site",
              "/root/.axon_site/_ro/trn_rl_repo"):
        if os.path.isdir(p) and p not in sys.path:
            sys.path.append(p)
    import concourse.bass as bass          # noqa: F401
    import concourse.tile as tile          # noqa: F401
    from concourse import mybir            # noqa: F401
    return bass, tile, mybir


def _build_graph():
    bass, tile, mybir = _import_concourse()
    from concourse import bacc
    f16 = mybir.dt.float16
    f32 = mybir.dt.float32
    AF = mybir.ActivationFunctionType
    ALU = mybir.AluOpType

    nc = bacc.Bacc("TRN2", target_bir_lowering=False, debug=False,
                   num_devices=N_CORES)

    xT = nc.declare_dram_parameter("xT", [IN_DIM, N_SHARD], f16, isOutput=False)
    wpack = nc.declare_dram_parameter("wpack", [128, WPACK_COLS], f16,
                                      isOutput=False)
    bpack = nc.declare_dram_parameter("bpack", [128, BPACK_COLS], f32,
                                      isOutput=False)
    out = nc.declare_dram_parameter("out", [3], f32, isOutput=True)

    xT_r = xT.rearrange("(ko ki) n -> ki ko n", ki=128)

    with tile.TileContext(nc) as tc:
        with (
            tc.tile_pool(name="singles", bufs=1) as singles,
            tc.tile_pool(name="xin", bufs=4) as xin,
            tc.tile_pool(name="h16p", bufs=3) as h16p,
            tc.tile_pool(name="gsb", bufs=2) as gsb,
            tc.tile_pool(name="gates", bufs=2) as gates,
            tc.tile_pool(name="wexp", bufs=2) as wexp,
            tc.tile_pool(name="acc", bufs=1) as accp,
            tc.tile_pool(name="ps_h", bufs=1, space="PSUM") as ps_h,
            tc.tile_pool(name="ps_av", bufs=1, space="PSUM") as ps_av,
            tc.tile_pool(name="ps_au", bufs=1, space="PSUM") as ps_au,
            tc.tile_pool(name="ps_lg", bufs=1, space="PSUM") as ps_lg,
            tc.tile_pool(name="ps_c", bufs=1, space="PSUM") as ps_c,
        ):
            # ---- preamble: two DMAs, engine-local bias staging ----
            wp = singles.tile([128, WPACK_COLS], f16)
            # W_pe lands in its own DMA so the first h matmul isn't gated on
            # the (later-needed) V/U/att/cls weights.
            half_pe = W_V_OFF // 2
            nc.sync.dma_start(out=wp[:, 0:half_pe], in_=wpack[:, 0:half_pe])
            nc.sync.dma_start(out=wp[:, half_pe:W_V_OFF],
                              in_=wpack[:, half_pe:W_V_OFF])
            nc.sync.dma_start(out=wp[:, W_V_OFF:], in_=wpack[:, W_V_OFF:])
            bias_sb = singles.tile([128, BPACK_COLS], f32)
            nc.sync.dma_start(out=bias_sb, in_=bpack[:, :])

            wpe = wp[:, W_PE_OFF:W_V_OFF].rearrange("p (ko e) -> p ko e", ko=KC)
            wv = wp[:, W_V_OFF:W_U_OFF].rearrange("p (mo a) -> p mo a", mo=MC)
            wu = wp[:, W_U_OFF:W_ATT_OFF].rearrange("p (mo a) -> p mo a", mo=MC)
            watt2 = wp[:, W_ATT_OFF:W_CLS_OFF]
            wcls = wp[:, W_CLS_OFF:WPACK_COLS].rearrange(
                "p (mo o) -> p mo o", mo=MC)

            # DVE-local biases (feed tensor_scalar_add)
            bpe_dve = singles.tile([128, MC], f32)
            nc.vector.tensor_copy(bpe_dve, bias_sb[:, 0:MC])
            # ACT-local biases (feed activation bias port)
            bpe_act = singles.tile([128, MC], f32)
            nc.scalar.activation(bpe_act, bias_sb[:, 0:MC], AF.Identity)
            bvu_act = singles.tile([128, 2], f32)
            nc.scalar.activation(bvu_act, bias_sb[:, 4:6], AF.Identity)
            batt_act = singles.tile([2, 1], f32)
            nc.scalar.activation(batt_act, bias_sb[0:2, 6:7], AF.Identity)

            acc_s = accp.tile([2, N_TILES], f32)
            acc_wc = accp.tile([2, N_TILES], f32)

            # Dummy matmul: makes PE observe the W_pe DMA semaphore early.
            warm_ps = ps_h.tile([128, 2, 512], f32, tag="h_ps")
            nc.tensor.matmul(warm_ps[:, 0, 0:1], lhsT=wp[:, 0:128],
                             rhs=wp[:, 0:1], start=True, stop=True)

            def emit_front(t):
                """DMA + patch-embed matmuls + h16 copies for tile t."""
                tsl = slice(t * TILE, (t + 1) * TILE)
                xt = xin.tile([128, KC, TILE], f16, tag="xt")
                for kk in range(0, KC, 2):
                    nc.sync.dma_start(out=xt[:, kk:kk + 2, :],
                                      in_=xT_r[:, kk:kk + 2, tsl])

                # hT in two emb-chunk pairs; ps_h bufs=2 double-buffers the
                # PSUM banks so pass (t,1) streams while pass (t,0) copies.
                h16 = h16p.tile([128, MC, TILE], f16, tag="h16")
                for half in range(2):
                    h_ps = ps_h.tile([128, 2, 512], f32, tag="h_ps")
                    # k-outer: the first matmuls only need the first x/weight
                    # chunks, so streaming starts before the tail chunks land
                    for k in range(KC):
                        for m2 in range(2):
                            m = 2 * half + m2
                            nc.tensor.matmul(
                                h_ps[:, m2, :TILE],
                                lhsT=wpe[:, k, m * 128:(m + 1) * 128],
                                rhs=xt[:, k, :],
                                start=(k == 0), stop=(k == KC - 1),
                            )
                    # PSUM f32 -> SBUF f16 with bias add, split across the
                    # two non-PE engines so the pair completes in one hop.
                    m0 = 2 * half
                    nc.scalar.activation(
                        h16[:, m0, :], h_ps[:, 0, :TILE], AF.Identity,
                        bias=bpe_act[:, m0:m0 + 1])
                    nc.vector.tensor_scalar_add(
                        h16[:, m0 + 1, :], h_ps[:, 1, :TILE],
                        bpe_dve[:, m0 + 1:m0 + 2])
                return h16

            def emit_back(t, h16):
                """Gates, logits, classifier and partial sums for tile t.
                Emitted one tile behind emit_front so the PE streams tile
                t+1's patch-embed while ACT/DVE finish tile t — the h16
                handoff never stalls the PE."""
                av_ps = ps_av.tile([128, 512], f32, tag="av")
                au_ps = ps_au.tile([128, 512], f32, tag="au")
                for m in range(MC):
                    nc.tensor.matmul(
                        av_ps[:, :TILE], lhsT=wv[:, m, :], rhs=h16[:, m, :],
                        start=(m == 0), stop=(m == MC - 1),
                    )
                    nc.tensor.matmul(
                        au_ps[:, :TILE], lhsT=wu[:, m, :], rhs=h16[:, m, :],
                        start=(m == 0), stop=(m == MC - 1),
                    )

                # ACT: gv = tanh(av + b_V); gu = tanh(au/2 + b_U/2)
                gv = gates.tile([128, TILE], f16, tag="gv")
                nc.scalar.activation(gv, av_ps[:, :TILE], AF.Tanh,
                                     bias=bvu_act[:, 0:1])
                gu = gates.tile([128, TILE], f16, tag="gu")
                nc.scalar.activation(gu, au_ps[:, :TILE], AF.Tanh,
                                     bias=bvu_act[:, 1:2], scale=0.5)
                # DVE: G' = (gu + 1) * gv   (= 2 * A_V * A_U)
                g16 = gates.tile([128, TILE], f16, tag="g16")
                nc.vector.scalar_tensor_tensor(
                    g16, gu, 1.0, gv, op0=ALU.add, op1=ALU.mult)

                # attention logits, duplicated onto 2 partitions (PE)
                lg_ps = ps_lg.tile([2, 512], f32, tag="lg")
                nc.tensor.matmul(lg_ps[:, :TILE], lhsT=watt2, rhs=g16,
                                 start=True, stop=True)
                # classifier (PE; needs only h16)
                c_ps = ps_c.tile([2, 512], f32, tag="c")
                for m in range(MC):
                    nc.tensor.matmul(
                        c_ps[:, :TILE], lhsT=wcls[:, m, :], rhs=h16[:, m, :],
                        start=(m == 0), stop=(m == MC - 1),
                    )

                # ACT: w = exp(logit + b_att); s partial = sum_n w.
                # NOTE: accum_out (ACT accumulator / tensor_tensor_reduce)
                # raises NRT_EXEC_UNIT_UNRECOVERABLE on this hardware path,
                # so the reductions run as separate DVE instructions.
                w2 = wexp.tile([2, TILE], f32, tag="w2")
                nc.scalar.activation(w2, lg_ps[:, :TILE], AF.Exp,
                                     bias=batt_act)
                nc.vector.reduce_sum(acc_s[:, t:t + 1], w2,
                                     axis=mybir.AxisListType.X)

                # DVE: wc partial = sum_n w * cT (c read straight from PSUM)
                wc_scr = wexp.tile([2, TILE], f32, tag="wc")
                nc.vector.tensor_mul(wc_scr, w2, c_ps[:, :TILE])
                nc.vector.reduce_sum(acc_wc[:, t:t + 1], wc_scr,
                                     axis=mybir.AxisListType.X)

            # two-deep skew: PE streams tile t's patch-embed while tile
            # t-2's gates/logits retire — ACT/DVE latency chains never
            # back-pressure the PE.
            hist = []
            for t in range(N_TILES + 2):
                cur = emit_front(t) if t < N_TILES else None
                hist.append(cur)
                if t >= 2:
                    emit_back(t - 2, hist[t - 2])

            # final reduction over tiles and writeback
            s_fin = accp.tile([2, 1], f32)
            nc.vector.reduce_sum(s_fin, acc_s, axis=mybir.AxisListType.X)
            wc_fin = accp.tile([2, 1], f32)
            nc.vector.reduce_sum(wc_fin, acc_wc, axis=mybir.AxisListType.X)
            nc.sync.dma_start(out=out[0:2], in_=wc_fin[:, 0:1])
            nc.sync.dma_start(out=out[2:3], in_=s_fin[0:1, 0:1])

    # Runs the bacc passes (move_matmul_waits_to_ldweights,
    # generate_event_semaphores, DCE, ...) that make the BIR satisfy the
    # hardware's one-sync-wait-per-instruction constraint.
    nc.compile()
    return nc


def _prep_in_maps(x, W_pe, b_pe, W_V, b_V, W_U, b_U, w_att, b_att, W_cls):
    f16 = np.float16
    f32 = np.float32

    wpack = np.empty((128, WPACK_COLS), dtype=f16)
    wpack[:, W_PE_OFF:W_V_OFF] = (
        W_pe.reshape(KC, 128, EMB).transpose(1, 0, 2).reshape(128, KC * EMB))
    wpack[:, W_V_OFF:W_U_OFF] = (
        W_V.reshape(MC, 128, ATT).transpose(1, 0, 2).reshape(128, MC * ATT))
    wpack[:, W_U_OFF:W_ATT_OFF] = (
        W_U.reshape(MC, 128, ATT).transpose(1, 0, 2).reshape(128, MC * ATT))
    # kernel computes G' = 2*A_V*A_U; 0.5*w_att undoes the factor of 2.
    # Duplicated to 2 columns so the logit matmul broadcasts w onto both
    # output partitions.
    wpack[:, W_ATT_OFF:W_CLS_OFF] = np.concatenate(
        [0.5 * w_att, 0.5 * w_att], axis=1)
    wpack[:, W_CLS_OFF:WPACK_COLS] = (
        W_cls.reshape(MC, 128, NCLS).transpose(1, 0, 2).reshape(128, MC * NCLS))

    bpack = np.empty((128, BPACK_COLS), dtype=f32)
    bpack[:, 0:MC] = b_pe.reshape(MC, 128).T
    bpack[:, 4] = b_V
    # kernel computes tanh(0.5*y + bias) for the U branch -> bias = b_U/2
    bpack[:, 5] = 0.5 * b_U
    bpack[:, 6] = b_att[0]

    shared = {"wpack": wpack, "bpack": bpack}
    in_maps = []
    half = N // 2
    for c in range(N_CORES):
        bag, hi = divmod(c, 2)
        xs = x[bag, hi * half:(hi + 1) * half, :]
        in_maps.append({"xT": xs.T.astype(f16), **shared})
    return in_maps


def _run(inputs, trace=False, tmpdir=None):
    _import_concourse()
    from concourse.bass_utils import run_bass_kernel_spmd

    if "nc" not in _cache:
        _cache["nc"] = _build_graph()
    nc = _cache["nc"]

    in_maps = _prep_in_maps(
        inputs["x"], inputs["W_pe"], inputs["b_pe"], inputs["W_V"],
        inputs["b_V"], inputs["W_U"], inputs["b_U"], inputs["w_att"],
        inputs["b_att"], inputs["W_cls"])

    res = run_bass_kernel_spmd(
        nc, in_maps, core_ids=list(range(N_CORES)),
        trace=trace, tmpdir=tmpdir)

    b_cls = np.asarray(inputs["b_cls"], dtype=np.float32)
    logits = np.zeros((B, NCLS), dtype=np.float32)
    for bag in range(B):
        r0 = res.results[2 * bag]["out"]
        r1 = res.results[2 * bag + 1]["out"]
        wc = r0[0:2] + r1[0:2]
        s = r0[2] + r1[2]
        logits[bag] = wc / s + b_cls
    return logits, res


def kernel(**inputs):
    logits, _ = _run(inputs, trace=False)
    return logits
